# revision 37
# baseline (speedup 1.0000x reference)
"""Trainium2 Bass kernel for nn_FAM (dynamic grouped 3x3 low-pass filter + frequency gating).

Data-parallel over batch: 16 images -> 8 cores x 2 images.

v6: bf16-resident x. Both images are cast fp32->bf16 during the load DMA
(SWDGE ring, descriptor generation on otherwise-idle GpSimd) into 33 ring
segments [128(h), 16ch*130(w)] bf16 -- the whole batch stays resident, so
image 1's load streams at full rate behind image 0's and its pooling
finishes during conv(0). Stores go on the ACT HWDGE ring, pooled-row
bounce DMAs on the (otherwise empty) SP ring.

Pooling runs on the PE: per segment, 4 selector matmuls (one-hot lhsT
column block) accumulate column-sums into PSUM quarters P_q[16, 512];
4 small DVE reduces collapse w -> S[16, 16] per half-image; a tiny DRAM
bounce flattens S into the pooled row [1, 256].

Per-core algorithm (per image):
  pooled[c] = PE column-sums + DVE w-reduce + DRAM bounce
  filt = tanh(BN(conv_w @ pooled))                   (PE + ACT tanh)
  G_dx[h,h'] = sum_dy filt[g,dy*3+dx]*delta(h'=reflect(h+dy-1))
               (DVE bf16, built lazily per group during conv)
  per 16-ch half-group, 4-channel matmul batches (N=512, all bf16):
     xs1  = (s1/s2)*x           (DVE bf16 4x mode)
     PSUM = sum_dx G_dx^T @ xs1_dxview + I^T @ x
     outst = s2*PSUM + beta     (split DVE / ACT)
where s1 = (ia+1)(ll+1)-(lh+1), s2 = lh+1, beta = -ia*(ll+1)*mean(x[c]).
"""

import os
import sys

for _p in ("/opt/trn_rl_repo", "/opt/pypackages"):
    if _p not in sys.path and os.path.isdir(_p):
        sys.path.append(_p)

from contextlib import ExitStack

import numpy as np

import concourse.bass as bass
import concourse.tile as tile
from concourse import bacc, mybir
from concourse.bass_utils import run_bass_kernel_spmd

F32 = mybir.dt.float32
BF16 = mybir.dt.bfloat16
AF = mybir.ActivationFunctionType
ALU = mybir.AluOpType

N_CORES = 8
N_PER_CORE = 2        # images per core
C = 256               # channels
G = 8                 # groups
CG = C // G           # 32 channels per group
H = W = 128
HW = H * W
K = 3
BN_EPS = 1e-5
HG_CH = 16            # channels per segment / half-group
N_HG = C // HG_CH     # 16 segments per image
WPAD = W + 2          # 130: col-padded row length per channel
SEG_BUFS = 33         # both images resident + 1 spare


def _reflect(i: int) -> int:
    if i < 0:
        return -i
    if i > H - 1:
        return 2 * (H - 1) - i
    return i


def _host_consts(conv_w, bn_gamma, bn_beta, bn_mean, bn_var, lamb_l, lamb_h, inside_all):
    """Host-side parameter prep (no x-dependent math)."""
    import ml_dtypes

    s_bn = bn_gamma / np.sqrt(bn_var + BN_EPS)
    bn_scale = (s_bn / HW).astype(np.float32)
    bn_bias = (bn_beta - bn_mean * s_bn).astype(np.float32)
    bnsb = np.stack([bn_scale, bn_bias], axis=1)          # [72, 2]

    s1 = (inside_all + 1.0) * (lamb_l + 1.0) - (lamb_h + 1.0)
    s2 = lamb_h + 1.0
    mb = -inside_all * (lamb_l + 1.0) / HW
    sbc = np.concatenate([s1 / s2, s2]).astype(np.float32)  # [512]
    sbc = np.broadcast_to(sbc[None, :], (128, 512)).copy()  # [128, 512]
    mbrow = mb.astype(np.float32).reshape(1, 256).copy()  # [1, 256]

    d_up = np.zeros((128, 128), np.float32)
    d_dn = np.zeros((128, 128), np.float32)
    idn = np.eye(128, dtype=np.float32)
    for h in range(H):
        d_up[_reflect(h - 1), h] = 1.0
        d_dn[_reflect(h + 1), h] = 1.0
    dmats = np.concatenate([d_up, idn, d_dn], axis=1)     # [128, 384]
    dmats_bf = dmats.astype(ml_dtypes.bfloat16)           # exact: 0/1 entries

    wt = conv_w.T.astype(np.float32)                      # [256, 72]
    wtd = np.concatenate([wt[:128], wt[128:]], axis=1)    # [128, 144]

    # selector blocks for PE pooling: block hg has column (hg % 8) all-ones
    sel = np.zeros((128, 256), np.float32)
    for hg in range(N_HG):
        sel[:, hg * 16 + (hg % 8)] = 1.0
    sel_bf = sel.astype(ml_dtypes.bfloat16)

    return dict(dmats_bf=dmats_bf, sbc=sbc, mbrow=mbrow,
                wtd=wtd, bnsb=bnsb, sel_bf=sel_bf), (
        (s1 / s2).astype(np.float32), s2.astype(np.float32))


def _build_kernel(ctx: ExitStack, tc: "tile.TileContext",
                  x_ap: bass.AP, out_ap: bass.AP, pb_ap: bass.AP,
                  dmats_bf_ap: bass.AP, sbc_ap: bass.AP,
                  mbrow_ap: bass.AP, wtd_ap: bass.AP, bnsb_ap: bass.AP,
                  sel_bf_ap: bass.AP, s1s2: np.ndarray, s2v: np.ndarray):
    nc = tc.nc

    cpool = ctx.enter_context(tc.tile_pool(name="consts", bufs=1))
    stpool = ctx.enter_context(tc.tile_pool(name="stats", bufs=1))
    segpool = ctx.enter_context(tc.tile_pool(name="seg", bufs=SEG_BUFS))
    xspool = ctx.enter_context(tc.tile_pool(name="xs1", bufs=3))
    opool = ctx.enter_context(tc.tile_pool(name="outst", bufs=2))
    mpsum = ctx.enter_context(tc.tile_pool(name="mpsum", bufs=4, space="PSUM"))
    ppsum = ctx.enter_context(tc.tile_pool(name="ppsum", bufs=4, space="PSUM"))

    # ---- constants to SBUF ----
    dmbf_sb = cpool.tile([128, 384], BF16)
    nc.sync.dma_start(dmbf_sb[:], dmats_bf_ap)
    sbc_sb = cpool.tile([128, 512], F32)
    nc.sync.dma_start(sbc_sb[:], sbc_ap)
    mbrow_sb = cpool.tile([1, 256], F32)
    nc.sync.dma_start(mbrow_sb[:], mbrow_ap)
    wtd_sb = cpool.tile([128, 144], F32)
    nc.sync.dma_start(wtd_sb[:], wtd_ap)
    bnsb_sb = cpool.tile([72, 2], F32)
    nc.sync.dma_start(bnsb_sb[:], bnsb_ap)
    selb_sb = cpool.tile([128, 256], BF16)
    nc.sync.dma_start(selb_sb[:], sel_bf_ap)
    ones_sb = cpool.tile([1, 128], F32)
    nc.vector.memset(ones_sb[:], 1.0)
    idn_f = cpool.tile([128, 128], F32)
    nc.vector.tensor_copy(idn_f[:], dmbf_sb[:, 128:256])

    idnb = dmbf_sb[:, 128:256]                            # [128,128] bf16 identity

    # persistent per-image tiles
    fbs, b_n, gt, prow, S_t = {}, {}, {}, {}, {}
    for n in range(N_PER_CORE):
        fbs[n] = stpool.tile([128, 72], F32, name=f"fbs_{n}")
        b_n[n] = stpool.tile([128, 256], F32, name=f"bn_{n}")
        gt[n] = stpool.tile([128, G * 3 * 128], BF16, name=f"gt_{n}")
        prow[n] = stpool.tile([1, 256], F32, name=f"prow_{n}")
        S_t[n] = {}
        for half in range(2):
            S_t[n][half] = stpool.tile([16, 16], F32, name=f"S_{n}_{half}")

    segs = {}   # (n, hg) -> seg tile AP
    pq = {}     # (n, half) -> list of 4 PSUM quarter tiles
    bias_t = {}  # n -> tanh bias carrying the half-A fpre contribution

    def pool_half_reduce(n, half):
        """Collapse the 4 PSUM pool quarters of a half-image into S[16, 16]."""
        S = S_t[n][half]
        for q in range(4):
            p3 = pq[(n, half)][q].rearrange("p (c w) -> p c w", c=4)
            nc.vector.tensor_reduce(
                out=S[:, q * 4:(q + 1) * 4], in_=p3,
                axis=mybir.AxisListType.X, op=ALU.add)
        # bounce rows 0..7 (= channels half*128 .. half*128+127) out to DRAM
        nc.sync.dma_start(
            pb_ap[n, half * 128:(half + 1) * 128].rearrange("(a b) -> a b", a=8),
            S[0:8, 0:16])

    def filt_half(n, b):
        """Per-half filt work: pooled column, fpre contribution, beta block.

        Half A (b=0) runs mid-load, right after segments 0-7 have pooled;
        half B completes the chain once the whole image is resident.
        """
        nc.sync.dma_start(prow[n][0:1, b * 128:(b + 1) * 128],
                          pb_ap[n:n + 1, b * 128:(b + 1) * 128])
        pcp = ppsum.tile([128, 1], F32, name="pcp", tag="pp")
        nc.tensor.transpose(pcp[:], prow[n][0:1, b * 128:(b + 1) * 128],
                            idn_f[0:1, 0:1])
        pcol = stpool.tile([128, 1], F32, name=f"pcol_{n}_{b}")
        nc.scalar.copy(pcol[:], pcp[:])
        fpre = ppsum.tile([72, 1], F32, name="fpre", tag="pp")
        nc.tensor.matmul(fpre[:], lhsT=wtd_sb[:, b * 72:(b + 1) * 72],
                         rhs=pcol[:], start=True, stop=True)
        if b == 0:
            # bias' = bn_scale * fpre_A + bn_bias, folded into the tanh bias
            bias_t[n] = stpool.tile([72, 1], F32, name=f"biasA_{n}")
            nc.vector.scalar_tensor_tensor(
                out=bias_t[n][:], in0=fpre[:], scalar=bnsb_sb[:, 0:1],
                in1=bnsb_sb[:, 1:2], op0=ALU.mult, op1=ALU.add)
        else:
            filt_sb = stpool.tile([72, 1], F32, name=f"filt_{n}")
            nc.scalar.activation(filt_sb[:], fpre[:], AF.Tanh,
                                 bias=bias_t[n][:], scale=bnsb_sb[:, 0:1])
            # transpose [72,1] -> [1,72], then broadcast to [128,72]
            ftp = ppsum.tile([1, 72], F32, name="ftp", tag="pp")
            nc.tensor.transpose(ftp[:], filt_sb[:], idn_f[0:72, 0:72])
            filt_row = stpool.tile([1, 72], F32, name=f"filtrow_{n}")
            nc.scalar.copy(filt_row[:], ftp[:])
            fbp = ppsum.tile([128, 72], F32, name="fbp", tag="pp")
            nc.tensor.matmul(fbp[:], lhsT=ones_sb[:], rhs=filt_row[:],
                             start=True, stop=True)
            nc.scalar.copy(fbs[n][:], fbp[:])
        # beta row -> broadcast to B_n [128, 256] for this half
        brow = stpool.tile([1, 128], F32, name=f"brow_{n}_{b}")
        nc.vector.tensor_tensor(brow[:], prow[n][0:1, b * 128:(b + 1) * 128],
                                mbrow_sb[0:1, b * 128:(b + 1) * 128], op=ALU.mult)
        bbp = ppsum.tile([128, 128], F32, name="bbp", tag="pp")
        nc.tensor.matmul(bbp[:], lhsT=ones_sb[:], rhs=brow[0:1, :],
                         start=True, stop=True)
        nc.scalar.copy(b_n[n][:, b * 128:(b + 1) * 128], bbp[:])

    def load_seg_dma(n, hg):
        """Issue just the cast-DMA for one segment (SWDGE ring)."""
        c0 = hg * HG_CH
        seg = segpool.tile([128, HG_CH * WPAD], BF16, name="seg", tag="seg")
        segs[(n, hg)] = seg
        s3 = seg.rearrange("p (c w) -> p c w", c=HG_CH)
        nc.gpsimd.dma_start(s3[:, :, 1:129],
                            x_ap[n, c0:c0 + HG_CH, :, :].transpose([1, 0, 2]))

    def post_seg(n, hg):
        """Edge-fix + PE pooling for a loaded segment; filt chain per half."""
        half = hg // 8
        s3 = segs[(n, hg)].rearrange("p (c w) -> p c w", c=HG_CH)
        nc.vector.tensor_copy(s3[:, :, 0:1], s3[:, :, 2:3])
        nc.vector.tensor_copy(s3[:, :, 129:130], s3[:, :, 127:128])
        if hg % 8 == 0:
            pq[(n, half)] = [
                ppsum.tile([16, 512], F32, name="pq", tag="pp")
                for _ in range(4)]
        for q in range(4):
            nc.tensor.matmul(
                pq[(n, half)][q][:],
                lhsT=selb_sb[:, hg * 16:(hg + 1) * 16],
                rhs=s3[:, q * 4:(q + 1) * 4, 1:129],
                start=(hg % 8 == 0), stop=(hg % 8 == 7))
        if hg % 8 == 7:
            pool_half_reduce(n, half)
            filt_half(n, half)

    def load_image(n):
        for hg in range(N_HG):
            load_seg_dma(n, hg)
            post_seg(n, hg)

    g_built = {n: set() for n in range(N_PER_CORE)}

    def g_build(n, g):
        # G_dx = f0*D_up + f1*I + f2*D_dn per (g, dx); reflect rows in D mats
        if g in g_built[n]:
            return
        g_built[n].add(g)
        for dx in range(3):
            blk = gt[n][:, (g * 3 + dx) * 128:(g * 3 + dx + 1) * 128]
            j0 = g * 9 + 0 * 3 + dx
            j1 = g * 9 + 1 * 3 + dx
            j2 = g * 9 + 2 * 3 + dx
            nc.vector.tensor_scalar(
                out=blk, in0=dmbf_sb[:, 0:128],
                scalar1=fbs[n][:, j0:j0 + 1], scalar2=None, op0=ALU.mult)
            nc.vector.scalar_tensor_tensor(
                out=blk, in0=dmbf_sb[:, 128:256],
                scalar=fbs[n][:, j1:j1 + 1], in1=blk,
                op0=ALU.mult, op1=ALU.add)
            nc.vector.scalar_tensor_tensor(
                out=blk, in0=dmbf_sb[:, 256:384],
                scalar=fbs[n][:, j2:j2 + 1], in1=blk,
                op0=ALU.mult, op1=ALU.add)

    def conv_image(n, hgs=None):
        for hg in (range(N_HG) if hgs is None else hgs):
            c0 = hg * HG_CH
            g = c0 // CG
            g_build(n, g)
            seg = segs.pop((n, hg))
            s3 = seg.rearrange("p (c w) -> p c w", c=HG_CH)
            outst = opool.tile([128, HG_CH * W], F32, name="outst")
            outst3 = outst.rearrange("p (c w) -> p c w", c=HG_CH)
            for q in range(4):
                xs1 = xspool.tile([128, 4 * WPAD], BF16, name="xs1")
                xs13 = xs1.rearrange("p (c w) -> p c w", c=4)
                for cc in range(4):
                    c = c0 + q * 4 + cc
                    ci = q * 4 + cc
                    nc.vector.tensor_scalar(
                        out=xs13[:, cc, :], in0=s3[:, ci, :],
                        scalar1=float(s1s2[c]), scalar2=None, op0=ALU.mult)
                ps = mpsum.tile([128, 512], F32, name="ps", tag="ps")
                for dx in range(3):
                    nc.tensor.matmul(
                        ps[:],
                        lhsT=gt[n][:, (g * 3 + dx) * 128:(g * 3 + dx + 1) * 128],
                        rhs=xs13[:, :, dx:dx + 128],
                        start=(dx == 0), stop=False)
                nc.tensor.matmul(
                    ps[:], lhsT=idnb,
                    rhs=s3[:, q * 4:(q + 1) * 4, 1:129],
                    start=False, stop=True)
                ps3 = ps.rearrange("p (c w) -> p c w", c=4)
                for cc in range(4):
                    ci = q * 4 + cc
                    c = c0 + ci
                    if ci % 16 in (0, 3, 6, 9, 12, 15):
                        nc.vector.tensor_scalar(
                            out=outst3[:, ci, :], in0=ps3[:, cc, :],
                            scalar1=float(s2v[c]),
                            scalar2=b_n[n][:, c:c + 1],
                            op0=ALU.mult, op1=ALU.add)
                    else:
                        nc.scalar.activation(
                            outst3[:, ci, :], ps3[:, cc, :], AF.Identity,
                            bias=b_n[n][:, c:c + 1],
                            scale=float(s2v[c]))
            nc.sync.dma_start(out_ap[n, c0:c0 + HG_CH, :, :].transpose([1, 0, 2]),
                              outst3[:, :, :])

    LAG = 3
    load_image(0)
    for hg in range(N_HG):
        load_seg_dma(1, hg)
    for hg in range(N_HG):
        conv_image(0, [hg])
        if hg >= LAG:
            post_seg(1, hg - LAG)
    for hg in range(N_HG - LAG, N_HG):
        post_seg(1, hg)
    conv_image(1)


def build_nc(s1s2, s2v):
    nc = bacc.Bacc("TRN2", target_bir_lowering=False, debug=False)
    x_h = nc.dram_tensor("x", [N_PER_CORE, C, H, W], F32, kind="ExternalInput")
    dmbf_h = nc.dram_tensor("dmats_bf", [128, 384], BF16, kind="ExternalInput")
    sbc_h = nc.dram_tensor("sbc", [128, 512], F32, kind="ExternalInput")
    mbrow_h = nc.dram_tensor("mbrow", [1, 256], F32, kind="ExternalInput")
    wtd_h = nc.dram_tensor("wtd", [128, 144], F32, kind="ExternalInput")
    bnsb_h = nc.dram_tensor("bnsb", [72, 2], F32, kind="ExternalInput")
    selb_h = nc.dram_tensor("sel_bf", [128, 256], BF16, kind="ExternalInput")
    pb_h = nc.dram_tensor("pbounce", [N_PER_CORE, 256], F32, kind="Internal")
    out_h = nc.dram_tensor("out", [N_PER_CORE, C, H, W], F32, kind="ExternalOutput")

    with tile.TileContext(nc) as tc:
        with ExitStack() as ctx:
            _build_kernel(ctx, tc, x_h.ap(), out_h.ap(), pb_h.ap(),
                          dmbf_h.ap(), sbc_h.ap(), mbrow_h.ap(), wtd_h.ap(),
                          bnsb_h.ap(), selb_h.ap(), s1s2, s2v)
    nc.compile()
    return nc


def kernel(x, conv_w, bn_gamma, bn_beta, bn_mean, bn_var, lamb_l, lamb_h,
           inside_all, _trace=False, _trace_kwargs=None):
    x = np.ascontiguousarray(x, dtype=np.float32)
    consts, (s1s2, s2v) = _host_consts(conv_w, bn_gamma, bn_beta, bn_mean,
                                       bn_var, lamb_l, lamb_h, inside_all)
    nc = build_nc(s1s2, s2v)
    in_maps = []
    for i in range(N_CORES):
        m = {"x": x[i * N_PER_CORE:(i + 1) * N_PER_CORE]}
        m.update(consts)
        in_maps.append(m)
    kw = {}
    if _trace:
        kw["trace"] = True
        if _trace_kwargs:
            kw.update(_trace_kwargs)
    res = run_bass_kernel_spmd(nc, in_maps, list(range(N_CORES)), **kw)
    out = np.concatenate([res.results[i]["out"] for i in range(N_CORES)], axis=0)
    if _trace:
        kernel.last_results = res
    return out


# revision 38
# speedup vs baseline: 1.1216x; 1.1216x over previous
"""Trainium2 Bass kernel for nn_FAM (dynamic grouped 3x3 low-pass filter + frequency gating).

Data-parallel over batch: 16 images -> 8 cores x 2 images.

v6: bf16-resident x. Both images are cast fp32->bf16 during the load DMA
(SWDGE ring, descriptor generation on otherwise-idle GpSimd) into 33 ring
segments [128(h), 16ch*130(w)] bf16 -- the whole batch stays resident, so
image 1's load streams at full rate behind image 0's and its pooling
finishes during conv(0). Stores go on the ACT HWDGE ring, pooled-row
bounce DMAs on the (otherwise empty) SP ring.

Pooling runs on the PE: per segment, 4 selector matmuls (one-hot lhsT
column block) accumulate column-sums into PSUM quarters P_q[16, 512];
4 small DVE reduces collapse w -> S[16, 16] per half-image; a tiny DRAM
bounce flattens S into the pooled row [1, 256].

Per-core algorithm (per image):
  pooled[c] = PE column-sums + DVE w-reduce + DRAM bounce
  filt = tanh(BN(conv_w @ pooled))                   (PE + ACT tanh)
  G_dx[h,h'] = sum_dy filt[g,dy*3+dx]*delta(h'=reflect(h+dy-1))
               (DVE bf16, built lazily per group during conv)
  per 16-ch half-group, 4-channel matmul batches (N=512, all bf16):
     xs1  = (s1/s2)*x           (DVE bf16 4x mode)
     PSUM = sum_dx G_dx^T @ xs1_dxview + I^T @ x
     outst = s2*PSUM + beta     (split DVE / ACT)
where s1 = (ia+1)(ll+1)-(lh+1), s2 = lh+1, beta = -ia*(ll+1)*mean(x[c]).
"""

import os
import sys

for _p in ("/opt/trn_rl_repo", "/opt/pypackages"):
    if _p not in sys.path and os.path.isdir(_p):
        sys.path.append(_p)

from contextlib import ExitStack

import numpy as np

import concourse.bass as bass
import concourse.tile as tile
from concourse import bacc, mybir
from concourse.bass_utils import run_bass_kernel_spmd

F32 = mybir.dt.float32
BF16 = mybir.dt.bfloat16
AF = mybir.ActivationFunctionType
ALU = mybir.AluOpType

N_CORES = 8
N_PER_CORE = 2        # images per core
C = 256               # channels
G = 8                 # groups
CG = C // G           # 32 channels per group
H = W = 128
HW = H * W
K = 3
BN_EPS = 1e-5
HG_CH = 16            # channels per segment / half-group
N_HG = C // HG_CH     # 16 segments per image
WPAD = W + 2          # 130: col-padded row length per channel
SEG_BUFS = 33         # both images resident + 1 spare


def _reflect(i: int) -> int:
    if i < 0:
        return -i
    if i > H - 1:
        return 2 * (H - 1) - i
    return i


def _host_consts(conv_w, bn_gamma, bn_beta, bn_mean, bn_var, lamb_l, lamb_h, inside_all):
    """Host-side parameter prep (no x-dependent math)."""
    import ml_dtypes

    s_bn = bn_gamma / np.sqrt(bn_var + BN_EPS)
    bn_scale = (s_bn / HW).astype(np.float32)
    bn_bias = (bn_beta - bn_mean * s_bn).astype(np.float32)
    bnsb = np.stack([bn_scale, bn_bias], axis=1)          # [72, 2]

    s1 = (inside_all + 1.0) * (lamb_l + 1.0) - (lamb_h + 1.0)
    s2 = lamb_h + 1.0
    mb = -inside_all * (lamb_l + 1.0) / HW
    sbc = np.concatenate([s1 / s2, s2]).astype(np.float32)  # [512]
    sbc = np.broadcast_to(sbc[None, :], (128, 512)).copy()  # [128, 512]
    mbrow = mb.astype(np.float32).reshape(1, 256).copy()  # [1, 256]

    d_up = np.zeros((128, 128), np.float32)
    d_dn = np.zeros((128, 128), np.float32)
    idn = np.eye(128, dtype=np.float32)
    for h in range(H):
        d_up[_reflect(h - 1), h] = 1.0
        d_dn[_reflect(h + 1), h] = 1.0
    dmats = np.concatenate([d_up, idn, d_dn], axis=1)     # [128, 384]
    dmats_bf = dmats.astype(ml_dtypes.bfloat16)           # exact: 0/1 entries

    wt = conv_w.T.astype(np.float32)                      # [256, 72]
    wtd = np.concatenate([wt[:128], wt[128:]], axis=1)    # [128, 144]

    # selector blocks for PE pooling: block hg has column (hg % 8) all-ones
    sel = np.zeros((128, 256), np.float32)
    for hg in range(N_HG):
        sel[:, hg * 16 + (hg % 8)] = 1.0
    sel_bf = sel.astype(ml_dtypes.bfloat16)

    return dict(dmats_bf=dmats_bf, sbc=sbc, mbrow=mbrow,
                wtd=wtd, bnsb=bnsb, sel_bf=sel_bf), (
        (s1 / s2).astype(np.float32), s2.astype(np.float32))


def _build_kernel(ctx: ExitStack, tc: "tile.TileContext",
                  x_ap: bass.AP, out_ap: bass.AP, pb_ap: bass.AP,
                  dmats_bf_ap: bass.AP, sbc_ap: bass.AP,
                  mbrow_ap: bass.AP, wtd_ap: bass.AP, bnsb_ap: bass.AP,
                  sel_bf_ap: bass.AP, s1s2: np.ndarray, s2v: np.ndarray):
    nc = tc.nc

    cpool = ctx.enter_context(tc.tile_pool(name="consts", bufs=1))
    stpool = ctx.enter_context(tc.tile_pool(name="stats", bufs=1))
    segpool = ctx.enter_context(tc.tile_pool(name="seg", bufs=SEG_BUFS))
    xspool = ctx.enter_context(tc.tile_pool(name="xs1", bufs=3))
    opool = ctx.enter_context(tc.tile_pool(name="outst", bufs=2))
    mpsum = ctx.enter_context(tc.tile_pool(name="mpsum", bufs=4, space="PSUM"))
    ppsum = ctx.enter_context(tc.tile_pool(name="ppsum", bufs=4, space="PSUM"))

    # ---- constants to SBUF ----
    dmbf_sb = cpool.tile([128, 384], BF16)
    nc.sync.dma_start(dmbf_sb[:], dmats_bf_ap)
    sbc_sb = cpool.tile([128, 512], F32)
    nc.sync.dma_start(sbc_sb[:], sbc_ap)
    mbrow_sb = cpool.tile([1, 256], F32)
    nc.sync.dma_start(mbrow_sb[:], mbrow_ap)
    wtd_sb = cpool.tile([128, 144], F32)
    nc.sync.dma_start(wtd_sb[:], wtd_ap)
    bnsb_sb = cpool.tile([72, 2], F32)
    nc.sync.dma_start(bnsb_sb[:], bnsb_ap)
    selb_sb = cpool.tile([128, 256], BF16)
    nc.sync.dma_start(selb_sb[:], sel_bf_ap)
    ones_sb = cpool.tile([1, 128], F32)
    nc.vector.memset(ones_sb[:], 1.0)
    idn_f = cpool.tile([128, 128], F32)
    nc.vector.tensor_copy(idn_f[:], dmbf_sb[:, 128:256])

    idnb = dmbf_sb[:, 128:256]                            # [128,128] bf16 identity

    # persistent per-image tiles
    fbs, b_n, gt, prow, S_t = {}, {}, {}, {}, {}
    for n in range(N_PER_CORE):
        fbs[n] = stpool.tile([128, 72], F32, name=f"fbs_{n}")
        b_n[n] = stpool.tile([128, 256], F32, name=f"bn_{n}")
        gt[n] = stpool.tile([128, G * 3 * 128], BF16, name=f"gt_{n}")
        prow[n] = stpool.tile([1, 256], F32, name=f"prow_{n}")
        S_t[n] = {}
        for half in range(2):
            S_t[n][half] = stpool.tile([16, 16], F32, name=f"S_{n}_{half}")

    segs = {}   # (n, hg) -> seg tile AP
    pq = {}     # (n, half) -> list of 4 PSUM quarter tiles
    bias_t = {}  # n -> tanh bias carrying the half-A fpre contribution

    def pool_half_reduce(n, half):
        """Collapse the 4 PSUM pool quarters of a half-image into S[16, 16]."""
        S = S_t[n][half]
        for q in range(4):
            p3 = pq[(n, half)][q].rearrange("p (c w) -> p c w", c=4)
            nc.vector.tensor_reduce(
                out=S[:, q * 4:(q + 1) * 4], in_=p3,
                axis=mybir.AxisListType.X, op=ALU.add)
        # bounce rows 0..7 (= channels half*128 .. half*128+127) out to DRAM
        nc.sync.dma_start(
            pb_ap[n, half * 128:(half + 1) * 128].rearrange("(a b) -> a b", a=8),
            S[0:8, 0:16])

    def filt_half(n, b):
        """Per-half filt work: pooled column, fpre contribution, beta block.

        Half A (b=0) runs mid-load, right after segments 0-7 have pooled;
        half B completes the chain once the whole image is resident.
        """
        nc.sync.dma_start(prow[n][0:1, b * 128:(b + 1) * 128],
                          pb_ap[n:n + 1, b * 128:(b + 1) * 128])
        pcp = ppsum.tile([128, 1], F32, name="pcp", tag="pp")
        nc.tensor.transpose(pcp[:], prow[n][0:1, b * 128:(b + 1) * 128],
                            idn_f[0:1, 0:1])
        pcol = stpool.tile([128, 1], F32, name=f"pcol_{n}_{b}")
        nc.scalar.copy(pcol[:], pcp[:])
        fpre = ppsum.tile([72, 1], F32, name="fpre", tag="pp")
        nc.tensor.matmul(fpre[:], lhsT=wtd_sb[:, b * 72:(b + 1) * 72],
                         rhs=pcol[:], start=True, stop=True)
        if b == 0:
            # bias' = bn_scale * fpre_A + bn_bias, folded into the tanh bias
            bias_t[n] = stpool.tile([72, 1], F32, name=f"biasA_{n}")
            nc.vector.scalar_tensor_tensor(
                out=bias_t[n][:], in0=fpre[:], scalar=bnsb_sb[:, 0:1],
                in1=bnsb_sb[:, 1:2], op0=ALU.mult, op1=ALU.add)
        else:
            filt_sb = stpool.tile([72, 1], F32, name=f"filt_{n}")
            nc.scalar.activation(filt_sb[:], fpre[:], AF.Tanh,
                                 bias=bias_t[n][:], scale=bnsb_sb[:, 0:1])
            # transpose [72,1] -> [1,72], then broadcast to [128,72]
            ftp = ppsum.tile([1, 72], F32, name="ftp", tag="pp")
            nc.tensor.transpose(ftp[:], filt_sb[:], idn_f[0:72, 0:72])
            filt_row = stpool.tile([1, 72], F32, name=f"filtrow_{n}")
            nc.scalar.copy(filt_row[:], ftp[:])
            fbp = ppsum.tile([128, 72], F32, name="fbp", tag="pp")
            nc.tensor.matmul(fbp[:], lhsT=ones_sb[:], rhs=filt_row[:],
                             start=True, stop=True)
            nc.scalar.copy(fbs[n][:], fbp[:])
        # beta row -> broadcast to B_n [128, 256] for this half
        brow = stpool.tile([1, 128], F32, name=f"brow_{n}_{b}")
        nc.vector.tensor_tensor(brow[:], prow[n][0:1, b * 128:(b + 1) * 128],
                                mbrow_sb[0:1, b * 128:(b + 1) * 128], op=ALU.mult)
        bbp = ppsum.tile([128, 128], F32, name="bbp", tag="pp")
        nc.tensor.matmul(bbp[:], lhsT=ones_sb[:], rhs=brow[0:1, :],
                         start=True, stop=True)
        nc.scalar.copy(b_n[n][:, b * 128:(b + 1) * 128], bbp[:])

    def load_seg_dma(n, hg):
        """Issue just the cast-DMA for one segment (SWDGE ring)."""
        c0 = hg * HG_CH
        seg = segpool.tile([128, HG_CH * WPAD], BF16, name="seg", tag="seg")
        segs[(n, hg)] = seg
        s3 = seg.rearrange("p (c w) -> p c w", c=HG_CH)
        nc.gpsimd.dma_start(s3[:, :, 1:129],
                            x_ap[n, c0:c0 + HG_CH, :, :].transpose([1, 0, 2]))

    def post_seg(n, hg):
        """Edge-fix + PE pooling for a loaded segment; filt chain per half."""
        half = hg // 8
        s3 = segs[(n, hg)].rearrange("p (c w) -> p c w", c=HG_CH)
        nc.vector.tensor_copy(s3[:, :, 0:1], s3[:, :, 2:3])
        nc.vector.tensor_copy(s3[:, :, 129:130], s3[:, :, 127:128])
        if hg % 8 == 0:
            pq[(n, half)] = [
                ppsum.tile([16, 512], F32, name="pq", tag="pp")
                for _ in range(4)]
        for q in range(4):
            nc.tensor.matmul(
                pq[(n, half)][q][:],
                lhsT=selb_sb[:, hg * 16:(hg + 1) * 16],
                rhs=s3[:, q * 4:(q + 1) * 4, 1:129],
                start=(hg % 8 == 0), stop=(hg % 8 == 7))
        if hg % 8 == 7:
            pool_half_reduce(n, half)
            filt_half(n, half)

    def load_image(n):
        for hg in range(N_HG):
            load_seg_dma(n, hg)
            post_seg(n, hg)

    g_built = {n: set() for n in range(N_PER_CORE)}

    def g_build(n, g):
        # G_dx = f0*D_up + f1*I + f2*D_dn per (g, dx); reflect rows in D mats
        if g in g_built[n]:
            return
        g_built[n].add(g)
        for dx in range(3):
            blk = gt[n][:, (g * 3 + dx) * 128:(g * 3 + dx + 1) * 128]
            j0 = g * 9 + 0 * 3 + dx
            j1 = g * 9 + 1 * 3 + dx
            j2 = g * 9 + 2 * 3 + dx
            nc.vector.tensor_scalar(
                out=blk, in0=dmbf_sb[:, 0:128],
                scalar1=fbs[n][:, j0:j0 + 1], scalar2=None, op0=ALU.mult)
            nc.vector.scalar_tensor_tensor(
                out=blk, in0=dmbf_sb[:, 128:256],
                scalar=fbs[n][:, j1:j1 + 1], in1=blk,
                op0=ALU.mult, op1=ALU.add)
            nc.vector.scalar_tensor_tensor(
                out=blk, in0=dmbf_sb[:, 256:384],
                scalar=fbs[n][:, j2:j2 + 1], in1=blk,
                op0=ALU.mult, op1=ALU.add)

    def conv_image(n, hgs=None):
        for hg in (range(N_HG) if hgs is None else hgs):
            c0 = hg * HG_CH
            g = c0 // CG
            g_build(n, g)
            seg = segs.pop((n, hg))
            s3 = seg.rearrange("p (c w) -> p c w", c=HG_CH)
            outst = opool.tile([128, HG_CH * W], F32, name="outst")
            outst3 = outst.rearrange("p (c w) -> p c w", c=HG_CH)
            for q in range(4):
                xs1 = xspool.tile([128, 4 * WPAD], BF16, name="xs1")
                xs13 = xs1.rearrange("p (c w) -> p c w", c=4)
                for cc in range(4):
                    c = c0 + q * 4 + cc
                    ci = q * 4 + cc
                    nc.vector.tensor_scalar(
                        out=xs13[:, cc, :], in0=s3[:, ci, :],
                        scalar1=float(s1s2[c]), scalar2=None, op0=ALU.mult)
                ps = mpsum.tile([128, 512], F32, name="ps", tag="ps")
                for dx in range(3):
                    nc.tensor.matmul(
                        ps[:],
                        lhsT=gt[n][:, (g * 3 + dx) * 128:(g * 3 + dx + 1) * 128],
                        rhs=xs13[:, :, dx:dx + 128],
                        start=(dx == 0), stop=False)
                nc.tensor.matmul(
                    ps[:], lhsT=idnb,
                    rhs=s3[:, q * 4:(q + 1) * 4, 1:129],
                    start=False, stop=True)
                ps3 = ps.rearrange("p (c w) -> p c w", c=4)
                for cc in range(4):
                    ci = q * 4 + cc
                    c = c0 + ci
                    if ci % 4 == 0:
                        nc.vector.tensor_scalar(
                            out=outst3[:, ci, :], in0=ps3[:, cc, :],
                            scalar1=float(s2v[c]),
                            scalar2=b_n[n][:, c:c + 1],
                            op0=ALU.mult, op1=ALU.add)
                    else:
                        nc.scalar.activation(
                            outst3[:, ci, :], ps3[:, cc, :], AF.Identity,
                            bias=b_n[n][:, c:c + 1],
                            scale=float(s2v[c]))
            nc.sync.dma_start(out_ap[n, c0:c0 + HG_CH, :, :].transpose([1, 0, 2]),
                              outst3[:, :, :])

    LAG = 3
    load_image(0)
    for hg in range(N_HG):
        load_seg_dma(1, hg)
    for hg in range(N_HG):
        conv_image(0, [hg])
        if hg >= LAG:
            post_seg(1, hg - LAG)
    for hg in range(N_HG - LAG, N_HG):
        post_seg(1, hg)
    conv_image(1)


def build_nc(s1s2, s2v):
    nc = bacc.Bacc("TRN2", target_bir_lowering=False, debug=False)
    x_h = nc.dram_tensor("x", [N_PER_CORE, C, H, W], F32, kind="ExternalInput")
    dmbf_h = nc.dram_tensor("dmats_bf", [128, 384], BF16, kind="ExternalInput")
    sbc_h = nc.dram_tensor("sbc", [128, 512], F32, kind="ExternalInput")
    mbrow_h = nc.dram_tensor("mbrow", [1, 256], F32, kind="ExternalInput")
    wtd_h = nc.dram_tensor("wtd", [128, 144], F32, kind="ExternalInput")
    bnsb_h = nc.dram_tensor("bnsb", [72, 2], F32, kind="ExternalInput")
    selb_h = nc.dram_tensor("sel_bf", [128, 256], BF16, kind="ExternalInput")
    pb_h = nc.dram_tensor("pbounce", [N_PER_CORE, 256], F32, kind="Internal")
    out_h = nc.dram_tensor("out", [N_PER_CORE, C, H, W], F32, kind="ExternalOutput")

    with tile.TileContext(nc) as tc:
        with ExitStack() as ctx:
            _build_kernel(ctx, tc, x_h.ap(), out_h.ap(), pb_h.ap(),
                          dmbf_h.ap(), sbc_h.ap(), mbrow_h.ap(), wtd_h.ap(),
                          bnsb_h.ap(), selb_h.ap(), s1s2, s2v)
    nc.compile()
    return nc


def kernel(x, conv_w, bn_gamma, bn_beta, bn_mean, bn_var, lamb_l, lamb_h,
           inside_all, _trace=False, _trace_kwargs=None):
    x = np.ascontiguousarray(x, dtype=np.float32)
    consts, (s1s2, s2v) = _host_consts(conv_w, bn_gamma, bn_beta, bn_mean,
                                       bn_var, lamb_l, lamb_h, inside_all)
    nc = build_nc(s1s2, s2v)
    in_maps = []
    for i in range(N_CORES):
        m = {"x": x[i * N_PER_CORE:(i + 1) * N_PER_CORE]}
        m.update(consts)
        in_maps.append(m)
    kw = {}
    if _trace:
        kw["trace"] = True
        if _trace_kwargs:
            kw.update(_trace_kwargs)
    res = run_bass_kernel_spmd(nc, in_maps, list(range(N_CORES)), **kw)
    out = np.concatenate([res.results[i]["out"] for i in range(N_CORES)], axis=0)
    if _trace:
        kernel.last_results = res
    return out


# revision 41
# speedup vs baseline: 1.2011x; 1.0709x over previous
"""Trainium2 Bass kernel for nn_FAM (dynamic grouped 3x3 low-pass filter + frequency gating).

Data-parallel over batch: 16 images -> 8 cores x 2 images.

v6: bf16-resident x. Both images are cast fp32->bf16 during the load DMA
(SWDGE ring, descriptor generation on otherwise-idle GpSimd) into 33 ring
segments [128(h), 16ch*130(w)] bf16 -- the whole batch stays resident, so
image 1's load streams at full rate behind image 0's and its pooling
finishes during conv(0). Stores go on the ACT HWDGE ring, pooled-row
bounce DMAs on the (otherwise empty) SP ring.

Pooling runs on the PE: per segment, 4 selector matmuls (one-hot lhsT
column block) accumulate column-sums into PSUM quarters P_q[16, 512];
4 small DVE reduces collapse w -> S[16, 16] per half-image; a tiny DRAM
bounce flattens S into the pooled row [1, 256].

Per-core algorithm (per image):
  pooled[c] = PE column-sums + DVE w-reduce + DRAM bounce
  filt = tanh(BN(conv_w @ pooled))                   (PE + ACT tanh)
  G_dx[h,h'] = sum_dy filt[g,dy*3+dx]*delta(h'=reflect(h+dy-1))
               (DVE bf16, built lazily per group during conv)
  per 16-ch half-group, 4-channel matmul batches (N=512, all bf16):
     xs1  = (s1/s2)*x           (DVE bf16 4x mode)
     PSUM = sum_dx G_dx^T @ xs1_dxview + I^T @ x
     outst = s2*PSUM + beta     (split DVE / ACT)
where s1 = (ia+1)(ll+1)-(lh+1), s2 = lh+1, beta = -ia*(ll+1)*mean(x[c]).
"""

import os
import sys

for _p in ("/opt/trn_rl_repo", "/opt/pypackages"):
    if _p not in sys.path and os.path.isdir(_p):
        sys.path.append(_p)

from contextlib import ExitStack

import numpy as np

import concourse.bass as bass
import concourse.tile as tile
from concourse import bacc, mybir
from concourse.bass_utils import run_bass_kernel_spmd

F32 = mybir.dt.float32
BF16 = mybir.dt.bfloat16
AF = mybir.ActivationFunctionType
ALU = mybir.AluOpType

N_CORES = 8
N_PER_CORE = 2        # images per core
C = 256               # channels
G = 8                 # groups
CG = C // G           # 32 channels per group
H = W = 128
HW = H * W
K = 3
BN_EPS = 1e-5
HG_CH = 16            # channels per segment / half-group
N_HG = C // HG_CH     # 16 segments per image
WPAD = W + 2          # 130: col-padded row length per channel
SEG_BUFS = 33         # both images resident + 1 spare


def _reflect(i: int) -> int:
    if i < 0:
        return -i
    if i > H - 1:
        return 2 * (H - 1) - i
    return i


def _host_consts(conv_w, bn_gamma, bn_beta, bn_mean, bn_var, lamb_l, lamb_h, inside_all):
    """Host-side parameter prep (no x-dependent math)."""
    import ml_dtypes

    s_bn = bn_gamma / np.sqrt(bn_var + BN_EPS)
    bn_scale = (s_bn / HW).astype(np.float32)
    bn_bias = (bn_beta - bn_mean * s_bn).astype(np.float32)
    bnsb = np.stack([bn_scale, bn_bias], axis=1)          # [72, 2]

    s1 = (inside_all + 1.0) * (lamb_l + 1.0) - (lamb_h + 1.0)
    s2 = lamb_h + 1.0
    mb = -inside_all * (lamb_l + 1.0) / HW
    sbc = np.concatenate([s1 / s2, s2]).astype(np.float32)  # [512]
    sbc = np.broadcast_to(sbc[None, :], (128, 512)).copy()  # [128, 512]
    mbrow = mb.astype(np.float32).reshape(1, 256).copy()  # [1, 256]

    d_up = np.zeros((128, 128), np.float32)
    d_dn = np.zeros((128, 128), np.float32)
    idn = np.eye(128, dtype=np.float32)
    for h in range(H):
        d_up[_reflect(h - 1), h] = 1.0
        d_dn[_reflect(h + 1), h] = 1.0
    dmats = np.concatenate([d_up, idn, d_dn], axis=1)     # [128, 384]
    dmats_bf = dmats.astype(ml_dtypes.bfloat16)           # exact: 0/1 entries

    wt = conv_w.T.astype(np.float32)                      # [256, 72]
    wtd = np.concatenate([wt[:128], wt[128:]], axis=1)    # [128, 144]

    # selector blocks for PE pooling: block hg has column (hg % 8) all-ones
    sel = np.zeros((128, 256), np.float32)
    for hg in range(N_HG):
        sel[:, hg * 16 + (hg % 8)] = 1.0
    sel_bf = sel.astype(ml_dtypes.bfloat16)

    return dict(dmats_bf=dmats_bf, sbc=sbc, mbrow=mbrow,
                wtd=wtd, bnsb=bnsb, sel_bf=sel_bf), (
        (s1 / s2).astype(np.float32), s2.astype(np.float32))


def _build_kernel(ctx: ExitStack, tc: "tile.TileContext",
                  x_ap: bass.AP, out_ap: bass.AP, pb_ap: bass.AP,
                  dmats_bf_ap: bass.AP, sbc_ap: bass.AP,
                  mbrow_ap: bass.AP, wtd_ap: bass.AP, bnsb_ap: bass.AP,
                  sel_bf_ap: bass.AP, s1s2: np.ndarray, s2v: np.ndarray):
    nc = tc.nc

    cpool = ctx.enter_context(tc.tile_pool(name="consts", bufs=1))
    stpool = ctx.enter_context(tc.tile_pool(name="stats", bufs=1))
    segpool = ctx.enter_context(tc.tile_pool(name="seg", bufs=SEG_BUFS))
    xspool = ctx.enter_context(tc.tile_pool(name="xs1", bufs=3))
    opool = ctx.enter_context(tc.tile_pool(name="outst", bufs=3))
    mpsum = ctx.enter_context(tc.tile_pool(name="mpsum", bufs=6, space="PSUM"))
    ppsum = ctx.enter_context(tc.tile_pool(name="ppsum", bufs=2, space="PSUM"))

    # ---- constants to SBUF ----
    dmbf_sb = cpool.tile([128, 384], BF16)
    nc.sync.dma_start(dmbf_sb[:], dmats_bf_ap)
    sbc_sb = cpool.tile([128, 512], F32)
    nc.sync.dma_start(sbc_sb[:], sbc_ap)
    mbrow_sb = cpool.tile([1, 256], F32)
    nc.sync.dma_start(mbrow_sb[:], mbrow_ap)
    wtd_sb = cpool.tile([128, 144], F32)
    nc.sync.dma_start(wtd_sb[:], wtd_ap)
    bnsb_sb = cpool.tile([72, 2], F32)
    nc.sync.dma_start(bnsb_sb[:], bnsb_ap)
    selb_sb = cpool.tile([128, 256], BF16)
    nc.sync.dma_start(selb_sb[:], sel_bf_ap)
    ones_sb = cpool.tile([1, 128], F32)
    nc.vector.memset(ones_sb[:], 1.0)
    idn_f = cpool.tile([128, 128], F32)
    nc.vector.tensor_copy(idn_f[:], dmbf_sb[:, 128:256])

    idnb = dmbf_sb[:, 128:256]                            # [128,128] bf16 identity

    # persistent per-image tiles
    fbs, b_n, gt, prow, S_t = {}, {}, {}, {}, {}
    for n in range(N_PER_CORE):
        fbs[n] = stpool.tile([128, 72], F32, name=f"fbs_{n}")
        b_n[n] = stpool.tile([128, 256], F32, name=f"bn_{n}")
        gt[n] = stpool.tile([128, G * 3 * 128], BF16, name=f"gt_{n}")
        prow[n] = stpool.tile([1, 256], F32, name=f"prow_{n}")
        S_t[n] = {}
        for half in range(2):
            S_t[n][half] = stpool.tile([16, 16], F32, name=f"S_{n}_{half}")

    segs = {}   # (n, hg) -> seg tile AP
    pq = {}     # (n, half) -> list of 4 PSUM quarter tiles
    bias_t = {}  # n -> tanh bias carrying the half-A fpre contribution

    def pool_pass_reduce(n, half, pas):
        """Collapse this pass's 2 PSUM pool quarters into S columns."""
        S = S_t[n][half]
        for i, q in enumerate((2 * pas, 2 * pas + 1)):
            p3 = pq[(n, half, pas)][i].rearrange("p (c w) -> p c w", c=4)
            nc.vector.tensor_reduce(
                out=S[:, q * 4:(q + 1) * 4], in_=p3,
                axis=mybir.AxisListType.X, op=ALU.add)

    def pool_pass2(n, half):
        """Second pooling pass: re-read the (resident) segments for cl 8-15,
        then bounce the completed S rows out to DRAM."""
        pq[(n, half, 1)] = [
            ppsum.tile([16, 512], F32, name="pq", tag="pp") for _ in range(2)]
        for hg in range(half * 8, half * 8 + 8):
            s3 = segs[(n, hg)].rearrange("p (c w) -> p c w", c=HG_CH)
            for i, q in enumerate((2, 3)):
                nc.tensor.matmul(
                    pq[(n, half, 1)][i][:],
                    lhsT=selb_sb[:, hg * 16:(hg + 1) * 16],
                    rhs=s3[:, q * 4:(q + 1) * 4, 1:129],
                    start=(hg % 8 == 0), stop=(hg % 8 == 7))
        pool_pass_reduce(n, half, 1)
        # bounce rows 0..7 (= channels half*128 .. half*128+127) out to DRAM
        S = S_t[n][half]
        nc.sync.dma_start(
            pb_ap[n, half * 128:(half + 1) * 128].rearrange("(a b) -> a b", a=8),
            S[0:8, 0:16])

    def filt_half(n, b):
        """Per-half filt work: pooled column, fpre contribution, beta block.

        Half A (b=0) runs mid-load, right after segments 0-7 have pooled;
        half B completes the chain once the whole image is resident.
        """
        nc.sync.dma_start(prow[n][0:1, b * 128:(b + 1) * 128],
                          pb_ap[n:n + 1, b * 128:(b + 1) * 128])
        pcp = ppsum.tile([128, 1], F32, name="pcp", tag="pp")
        nc.tensor.transpose(pcp[:], prow[n][0:1, b * 128:(b + 1) * 128],
                            idn_f[0:1, 0:1])
        pcol = stpool.tile([128, 1], F32, name=f"pcol_{n}_{b}")
        nc.scalar.copy(pcol[:], pcp[:])
        fpre = ppsum.tile([72, 1], F32, name="fpre", tag="pp")
        nc.tensor.matmul(fpre[:], lhsT=wtd_sb[:, b * 72:(b + 1) * 72],
                         rhs=pcol[:], start=True, stop=True)
        if b == 0:
            # bias' = bn_scale * fpre_A + bn_bias, folded into the tanh bias
            bias_t[n] = stpool.tile([72, 1], F32, name=f"biasA_{n}")
            nc.vector.scalar_tensor_tensor(
                out=bias_t[n][:], in0=fpre[:], scalar=bnsb_sb[:, 0:1],
                in1=bnsb_sb[:, 1:2], op0=ALU.mult, op1=ALU.add)
        else:
            filt_sb = stpool.tile([72, 1], F32, name=f"filt_{n}")
            nc.scalar.activation(filt_sb[:], fpre[:], AF.Tanh,
                                 bias=bias_t[n][:], scale=bnsb_sb[:, 0:1])
            # transpose [72,1] -> [1,72], then broadcast to [128,72]
            ftp = ppsum.tile([1, 72], F32, name="ftp", tag="pp")
            nc.tensor.transpose(ftp[:], filt_sb[:], idn_f[0:72, 0:72])
            filt_row = stpool.tile([1, 72], F32, name=f"filtrow_{n}")
            nc.scalar.copy(filt_row[:], ftp[:])
            fbp = ppsum.tile([128, 72], F32, name="fbp", tag="pp")
            nc.tensor.matmul(fbp[:], lhsT=ones_sb[:], rhs=filt_row[:],
                             start=True, stop=True)
            nc.scalar.copy(fbs[n][:], fbp[:])
        # beta row -> broadcast to B_n [128, 256] for this half
        brow = stpool.tile([1, 128], F32, name=f"brow_{n}_{b}")
        nc.vector.tensor_tensor(brow[:], prow[n][0:1, b * 128:(b + 1) * 128],
                                mbrow_sb[0:1, b * 128:(b + 1) * 128], op=ALU.mult)
        bbp = ppsum.tile([128, 128], F32, name="bbp", tag="pp")
        nc.tensor.matmul(bbp[:], lhsT=ones_sb[:], rhs=brow[0:1, :],
                         start=True, stop=True)
        nc.scalar.copy(b_n[n][:, b * 128:(b + 1) * 128], bbp[:])

    def load_seg_dma(n, hg):
        """Issue just the cast-DMA for one segment (SWDGE ring)."""
        c0 = hg * HG_CH
        seg = segpool.tile([128, HG_CH * WPAD], BF16, name="seg", tag="seg")
        segs[(n, hg)] = seg
        s3 = seg.rearrange("p (c w) -> p c w", c=HG_CH)
        nc.gpsimd.dma_start(s3[:, :, 1:129],
                            x_ap[n, c0:c0 + HG_CH, :, :].transpose([1, 0, 2]))

    def post_seg(n, hg):
        """Edge-fix + PE pooling (pass 1) for a loaded segment."""
        half = hg // 8
        s3 = segs[(n, hg)].rearrange("p (c w) -> p c w", c=HG_CH)
        nc.vector.tensor_copy(s3[:, :, 0:1], s3[:, :, 2:3])
        nc.vector.tensor_copy(s3[:, :, 129:130], s3[:, :, 127:128])
        if hg % 8 == 0:
            pq[(n, half, 0)] = [
                ppsum.tile([16, 512], F32, name="pq", tag="pp")
                for _ in range(2)]
        for i, q in enumerate((0, 1)):
            nc.tensor.matmul(
                pq[(n, half, 0)][i][:],
                lhsT=selb_sb[:, hg * 16:(hg + 1) * 16],
                rhs=s3[:, q * 4:(q + 1) * 4, 1:129],
                start=(hg % 8 == 0), stop=(hg % 8 == 7))
        if hg % 8 == 7:
            pool_pass_reduce(n, half, 0)
            pool_pass2(n, half)
            filt_half(n, half)

    def load_image(n):
        for hg in range(N_HG):
            load_seg_dma(n, hg)
            post_seg(n, hg)

    g_built = {n: set() for n in range(N_PER_CORE)}

    def g_build(n, g):
        # G_dx = f0*D_up + f1*I + f2*D_dn per (g, dx); reflect rows in D mats
        if g in g_built[n]:
            return
        g_built[n].add(g)
        for dx in range(3):
            blk = gt[n][:, (g * 3 + dx) * 128:(g * 3 + dx + 1) * 128]
            j0 = g * 9 + 0 * 3 + dx
            j1 = g * 9 + 1 * 3 + dx
            j2 = g * 9 + 2 * 3 + dx
            nc.vector.tensor_scalar(
                out=blk, in0=dmbf_sb[:, 0:128],
                scalar1=fbs[n][:, j0:j0 + 1], scalar2=None, op0=ALU.mult)
            nc.vector.scalar_tensor_tensor(
                out=blk, in0=dmbf_sb[:, 128:256],
                scalar=fbs[n][:, j1:j1 + 1], in1=blk,
                op0=ALU.mult, op1=ALU.add)
            nc.vector.scalar_tensor_tensor(
                out=blk, in0=dmbf_sb[:, 256:384],
                scalar=fbs[n][:, j2:j2 + 1], in1=blk,
                op0=ALU.mult, op1=ALU.add)

    def conv_image(n, hgs=None):
        for hg in (range(N_HG) if hgs is None else hgs):
            c0 = hg * HG_CH
            g = c0 // CG
            g_build(n, g)
            seg = segs.pop((n, hg))
            s3 = seg.rearrange("p (c w) -> p c w", c=HG_CH)
            outst = opool.tile([128, HG_CH * W], F32, name="outst")
            outst3 = outst.rearrange("p (c w) -> p c w", c=HG_CH)
            for q in range(4):
                xs1 = xspool.tile([128, 4 * WPAD], BF16, name="xs1")
                xs13 = xs1.rearrange("p (c w) -> p c w", c=4)
                for cc in range(4):
                    c = c0 + q * 4 + cc
                    ci = q * 4 + cc
                    nc.vector.tensor_scalar(
                        out=xs13[:, cc, :], in0=s3[:, ci, :],
                        scalar1=float(s1s2[c]), scalar2=None, op0=ALU.mult)
                ps = mpsum.tile([128, 512], F32, name="ps", tag="ps")
                for dx in range(3):
                    nc.tensor.matmul(
                        ps[:],
                        lhsT=gt[n][:, (g * 3 + dx) * 128:(g * 3 + dx + 1) * 128],
                        rhs=xs13[:, :, dx:dx + 128],
                        start=(dx == 0), stop=False)
                nc.tensor.matmul(
                    ps[:], lhsT=idnb,
                    rhs=s3[:, q * 4:(q + 1) * 4, 1:129],
                    start=False, stop=True)
                ps3 = ps.rearrange("p (c w) -> p c w", c=4)
                for cc in range(4):
                    ci = q * 4 + cc
                    c = c0 + ci
                    if ci % 4 == 0:
                        nc.vector.tensor_scalar(
                            out=outst3[:, ci, :], in0=ps3[:, cc, :],
                            scalar1=float(s2v[c]),
                            scalar2=b_n[n][:, c:c + 1],
                            op0=ALU.mult, op1=ALU.add)
                    else:
                        nc.scalar.activation(
                            outst3[:, ci, :], ps3[:, cc, :], AF.Identity,
                            bias=b_n[n][:, c:c + 1],
                            scale=float(s2v[c]))
            nc.sync.dma_start(out_ap[n, c0:c0 + HG_CH, :, :].transpose([1, 0, 2]),
                              outst3[:, :, :])

    LAG = 3
    load_image(0)
    for hg in range(N_HG):
        load_seg_dma(1, hg)
    for hg in range(N_HG):
        conv_image(0, [hg])
        if hg >= LAG:
            post_seg(1, hg - LAG)
    for hg in range(N_HG - LAG, N_HG):
        post_seg(1, hg)
    conv_image(1)


def build_nc(s1s2, s2v):
    nc = bacc.Bacc("TRN2", target_bir_lowering=False, debug=False)
    x_h = nc.dram_tensor("x", [N_PER_CORE, C, H, W], F32, kind="ExternalInput")
    dmbf_h = nc.dram_tensor("dmats_bf", [128, 384], BF16, kind="ExternalInput")
    sbc_h = nc.dram_tensor("sbc", [128, 512], F32, kind="ExternalInput")
    mbrow_h = nc.dram_tensor("mbrow", [1, 256], F32, kind="ExternalInput")
    wtd_h = nc.dram_tensor("wtd", [128, 144], F32, kind="ExternalInput")
    bnsb_h = nc.dram_tensor("bnsb", [72, 2], F32, kind="ExternalInput")
    selb_h = nc.dram_tensor("sel_bf", [128, 256], BF16, kind="ExternalInput")
    pb_h = nc.dram_tensor("pbounce", [N_PER_CORE, 256], F32, kind="Internal")
    out_h = nc.dram_tensor("out", [N_PER_CORE, C, H, W], F32, kind="ExternalOutput")

    with tile.TileContext(nc) as tc:
        with ExitStack() as ctx:
            _build_kernel(ctx, tc, x_h.ap(), out_h.ap(), pb_h.ap(),
                          dmbf_h.ap(), sbc_h.ap(), mbrow_h.ap(), wtd_h.ap(),
                          bnsb_h.ap(), selb_h.ap(), s1s2, s2v)
    nc.compile()
    return nc


def kernel(x, conv_w, bn_gamma, bn_beta, bn_mean, bn_var, lamb_l, lamb_h,
           inside_all, _trace=False, _trace_kwargs=None):
    x = np.ascontiguousarray(x, dtype=np.float32)
    consts, (s1s2, s2v) = _host_consts(conv_w, bn_gamma, bn_beta, bn_mean,
                                       bn_var, lamb_l, lamb_h, inside_all)
    nc = build_nc(s1s2, s2v)
    in_maps = []
    for i in range(N_CORES):
        m = {"x": x[i * N_PER_CORE:(i + 1) * N_PER_CORE]}
        m.update(consts)
        in_maps.append(m)
    kw = {}
    if _trace:
        kw["trace"] = True
        if _trace_kwargs:
            kw.update(_trace_kwargs)
    res = run_bass_kernel_spmd(nc, in_maps, list(range(N_CORES)), **kw)
    out = np.concatenate([res.results[i]["out"] for i in range(N_CORES)], axis=0)
    if _trace:
        kernel.last_results = res
    return out


# revision 49
# speedup vs baseline: 1.2146x; 1.0112x over previous
"""Trainium2 Bass kernel for nn_FAM (dynamic grouped 3x3 low-pass filter + frequency gating).

Data-parallel over batch: 16 images -> 8 cores x 2 images.

v6: bf16-resident x. Both images are cast fp32->bf16 during the load DMA
(SWDGE ring, descriptor generation on otherwise-idle GpSimd) into 33 ring
segments [128(h), 16ch*130(w)] bf16 -- the whole batch stays resident, so
image 1's load streams at full rate behind image 0's and its pooling
finishes during conv(0). Stores go on the ACT HWDGE ring, pooled-row
bounce DMAs on the (otherwise empty) SP ring.

Pooling runs on the PE: per segment, 4 selector matmuls (one-hot lhsT
column block) accumulate column-sums into PSUM quarters P_q[16, 512];
4 small DVE reduces collapse w -> S[16, 16] per half-image; a tiny DRAM
bounce flattens S into the pooled row [1, 256].

Per-core algorithm (per image):
  pooled[c] = PE column-sums + DVE w-reduce + DRAM bounce
  filt = tanh(BN(conv_w @ pooled))                   (PE + ACT tanh)
  G_dx[h,h'] = sum_dy filt[g,dy*3+dx]*delta(h'=reflect(h+dy-1))
               (DVE bf16, built lazily per group during conv)
  per 16-ch half-group, 4-channel matmul batches (N=512, all bf16):
     xs1  = (s1/s2)*x           (DVE bf16 4x mode)
     PSUM = sum_dx G_dx^T @ xs1_dxview + I^T @ x
     outst = s2*PSUM + beta     (split DVE / ACT)
where s1 = (ia+1)(ll+1)-(lh+1), s2 = lh+1, beta = -ia*(ll+1)*mean(x[c]).
"""

import os
import sys

for _p in ("/opt/trn_rl_repo", "/opt/pypackages"):
    if _p not in sys.path and os.path.isdir(_p):
        sys.path.append(_p)

from contextlib import ExitStack

import numpy as np

import bass_rust
import concourse.bass as bass
import concourse.tile as tile
from concourse import bacc, mybir
from concourse.bass_utils import run_bass_kernel_spmd

F32 = mybir.dt.float32
BF16 = mybir.dt.bfloat16
AF = mybir.ActivationFunctionType
ALU = mybir.AluOpType

N_CORES = 8
N_PER_CORE = 2        # images per core
C = 256               # channels
G = 8                 # groups
CG = C // G           # 32 channels per group
H = W = 128
HW = H * W
K = 3
BN_EPS = 1e-5
HG_CH = 16            # channels per segment / half-group
N_HG = C // HG_CH     # 16 segments per image
WPAD = W + 2          # 130: col-padded row length per channel
SEG_BUFS = 33         # both images resident + 1 spare


def _reflect(i: int) -> int:
    if i < 0:
        return -i
    if i > H - 1:
        return 2 * (H - 1) - i
    return i


def _host_consts(conv_w, bn_gamma, bn_beta, bn_mean, bn_var, lamb_l, lamb_h, inside_all):
    """Host-side parameter prep (no x-dependent math)."""
    import ml_dtypes

    s_bn = bn_gamma / np.sqrt(bn_var + BN_EPS)
    bn_scale = (s_bn / HW).astype(np.float32)
    bn_bias = (bn_beta - bn_mean * s_bn).astype(np.float32)
    bnsb = np.stack([bn_scale, bn_bias], axis=1)          # [72, 2]

    s1 = (inside_all + 1.0) * (lamb_l + 1.0) - (lamb_h + 1.0)
    s2 = lamb_h + 1.0
    mb = -inside_all * (lamb_l + 1.0) / HW
    sbc = np.concatenate([s1 / s2, s2]).astype(np.float32)  # [512]
    sbc = np.broadcast_to(sbc[None, :], (128, 512)).copy()  # [128, 512]
    mbrow = mb.astype(np.float32).reshape(1, 256).copy()  # [1, 256]

    d_up = np.zeros((128, 128), np.float32)
    d_dn = np.zeros((128, 128), np.float32)
    idn = np.eye(128, dtype=np.float32)
    for h in range(H):
        d_up[_reflect(h - 1), h] = 1.0
        d_dn[_reflect(h + 1), h] = 1.0
    dmats = np.concatenate([d_up, idn, d_dn], axis=1)     # [128, 384]
    dmats_bf = dmats.astype(ml_dtypes.bfloat16)           # exact: 0/1 entries

    wt = conv_w.T.astype(np.float32)                      # [256, 72]
    wtd = np.concatenate([wt[:128], wt[128:]], axis=1)    # [128, 144]

    # selector blocks for PE pooling: block hg has column (hg % 8) all-ones
    sel = np.zeros((128, 256), np.float32)
    for hg in range(N_HG):
        sel[:, hg * 16 + (hg % 8)] = 1.0
    sel_bf = sel.astype(ml_dtypes.bfloat16)

    return dict(dmats_bf=dmats_bf, sbc=sbc, mbrow=mbrow,
                wtd=wtd, bnsb=bnsb, sel_bf=sel_bf), (
        (s1 / s2).astype(np.float32), s2.astype(np.float32))


def _build_kernel(ctx: ExitStack, tc: "tile.TileContext",
                  x_ap: bass.AP, out_ap: bass.AP, pb_ap: bass.AP,
                  dmats_bf_ap: bass.AP, sbc_ap: bass.AP,
                  mbrow_ap: bass.AP, wtd_ap: bass.AP, bnsb_ap: bass.AP,
                  sel_bf_ap: bass.AP, s1s2: np.ndarray, s2v: np.ndarray):
    nc = tc.nc

    cpool = ctx.enter_context(tc.tile_pool(name="consts", bufs=1))
    stpool = ctx.enter_context(tc.tile_pool(name="stats", bufs=1))
    segpool = ctx.enter_context(tc.tile_pool(name="seg", bufs=SEG_BUFS))
    xspool = ctx.enter_context(tc.tile_pool(name="xs1", bufs=5))
    opool = ctx.enter_context(tc.tile_pool(name="outst", bufs=3))
    mpsum = ctx.enter_context(tc.tile_pool(name="mpsum", bufs=5, space="PSUM"))
    ppsum = ctx.enter_context(tc.tile_pool(name="ppsum", bufs=3, space="PSUM"))

    # ---- constants to SBUF ----
    dmbf_sb = cpool.tile([128, 384], BF16)
    nc.sync.dma_start(dmbf_sb[:], dmats_bf_ap)
    sbc_sb = cpool.tile([128, 512], F32)
    nc.sync.dma_start(sbc_sb[:], sbc_ap)
    mbrow_sb = cpool.tile([1, 256], F32)
    nc.sync.dma_start(mbrow_sb[:], mbrow_ap)
    wtd_sb = cpool.tile([128, 144], F32)
    nc.sync.dma_start(wtd_sb[:], wtd_ap)
    bnsb_sb = cpool.tile([72, 2], F32)
    nc.sync.dma_start(bnsb_sb[:], bnsb_ap)
    selb_sb = cpool.tile([128, 256], BF16)
    nc.sync.dma_start(selb_sb[:], sel_bf_ap)
    ones_sb = cpool.tile([1, 128], F32)
    nc.vector.memset(ones_sb[:], 1.0)
    idn_f = cpool.tile([128, 128], F32)
    nc.vector.tensor_copy(idn_f[:], dmbf_sb[:, 128:256])

    idnb = dmbf_sb[:, 128:256]                            # [128,128] bf16 identity

    # persistent per-image tiles
    fbs, b_n, gt, prow, S_t = {}, {}, {}, {}, {}
    for n in range(N_PER_CORE):
        fbs[n] = stpool.tile([128, 72], F32, name=f"fbs_{n}")
        b_n[n] = stpool.tile([128, 256], F32, name=f"bn_{n}")
        gt[n] = stpool.tile([128, G * 3 * 128], BF16, name=f"gt_{n}")
        prow[n] = stpool.tile([1, 256], F32, name=f"prow_{n}")
        S_t[n] = {}
        for half in range(2):
            S_t[n][half] = stpool.tile([16, 16], F32, name=f"S_{n}_{half}")

    segs = {}   # (n, hg) -> seg tile AP
    pq = {}     # (n, half) -> list of 4 PSUM quarter tiles
    bias_t = {}  # n -> tanh bias carrying the half-A fpre contribution

    P1Q = (0, 1, 2)       # pooling pass-1 quarters (accumulated during load)
    P2Q = (3,)            # pass-2 quarters (re-read resident segs after load)

    def pool_pass_reduce(n, half, pas):
        """Collapse this pass's PSUM pool quarters into S columns."""
        S = S_t[n][half]
        for i, q in enumerate(P1Q if pas == 0 else P2Q):
            p3 = pq[(n, half, pas)][i].rearrange("p (c w) -> p c w", c=4)
            nc.vector.tensor_reduce(
                out=S[:, q * 4:(q + 1) * 4], in_=p3,
                axis=mybir.AxisListType.X, op=ALU.add)

    def pool_pass2(n, half):
        """Second pooling pass: re-read the (resident) segments for the last
        quarter, then bounce the completed S rows out to DRAM."""
        pq[(n, half, 1)] = [
            ppsum.tile([16, 512], F32, name="pq", tag="pp") for _ in P2Q]
        for hg in range(half * 8, half * 8 + 8):
            s3 = segs[(n, hg)].rearrange("p (c w) -> p c w", c=HG_CH)
            for i, q in enumerate(P2Q):
                nc.tensor.matmul(
                    pq[(n, half, 1)][i][:],
                    lhsT=selb_sb[:, hg * 16:(hg + 1) * 16],
                    rhs=s3[:, q * 4:(q + 1) * 4, 1:129],
                    start=(hg % 8 == 0), stop=(hg % 8 == 7))
        pool_pass_reduce(n, half, 1)
        # bounce rows 0..7 (= channels half*128 .. half*128+127) out to DRAM
        S = S_t[n][half]
        nc.sync.dma_start(
            pb_ap[n, half * 128:(half + 1) * 128].rearrange("(a b) -> a b", a=8),
            S[0:8, 0:16])

    def filt_half(n, b):
        """Per-half filt work: pooled column, fpre contribution, beta block.

        Half A (b=0) runs mid-load, right after segments 0-7 have pooled;
        half B completes the chain once the whole image is resident.
        """
        nc.sync.dma_start(prow[n][0:1, b * 128:(b + 1) * 128],
                          pb_ap[n:n + 1, b * 128:(b + 1) * 128])
        pcp = ppsum.tile([128, 1], F32, name="pcp", tag="pp")
        nc.tensor.transpose(pcp[:], prow[n][0:1, b * 128:(b + 1) * 128],
                            idn_f[0:1, 0:1])
        pcol = stpool.tile([128, 1], F32, name=f"pcol_{n}_{b}")
        nc.scalar.copy(pcol[:], pcp[:])
        fpre = ppsum.tile([72, 1], F32, name="fpre", tag="pp")
        nc.tensor.matmul(fpre[:], lhsT=wtd_sb[:, b * 72:(b + 1) * 72],
                         rhs=pcol[:], start=True, stop=True)
        if b == 0:
            # bias' = bn_scale * fpre_A + bn_bias, folded into the tanh bias
            bias_t[n] = stpool.tile([72, 1], F32, name=f"biasA_{n}")
            nc.vector.scalar_tensor_tensor(
                out=bias_t[n][:], in0=fpre[:], scalar=bnsb_sb[:, 0:1],
                in1=bnsb_sb[:, 1:2], op0=ALU.mult, op1=ALU.add)
        else:
            filt_sb = stpool.tile([72, 1], F32, name=f"filt_{n}")
            nc.scalar.activation(filt_sb[:], fpre[:], AF.Tanh,
                                 bias=bias_t[n][:], scale=bnsb_sb[:, 0:1])
            # transpose [72,1] -> [1,72], then broadcast to [128,72]
            ftp = ppsum.tile([1, 72], F32, name="ftp", tag="pp")
            nc.tensor.transpose(ftp[:], filt_sb[:], idn_f[0:72, 0:72])
            filt_row = stpool.tile([1, 72], F32, name=f"filtrow_{n}")
            nc.scalar.copy(filt_row[:], ftp[:])
            fbp = ppsum.tile([128, 72], F32, name="fbp", tag="pp")
            nc.tensor.matmul(fbp[:], lhsT=ones_sb[:], rhs=filt_row[:],
                             start=True, stop=True)
            nc.scalar.copy(fbs[n][:], fbp[:])
        # beta row -> broadcast to B_n [128, 256] for this half
        brow = stpool.tile([1, 128], F32, name=f"brow_{n}_{b}")
        nc.vector.tensor_tensor(brow[:], prow[n][0:1, b * 128:(b + 1) * 128],
                                mbrow_sb[0:1, b * 128:(b + 1) * 128], op=ALU.mult)
        bbp = ppsum.tile([128, 128], F32, name="bbp", tag="pp")
        nc.tensor.matmul(bbp[:], lhsT=ones_sb[:], rhs=brow[0:1, :],
                         start=True, stop=True)
        nc.scalar.copy(b_n[n][:, b * 128:(b + 1) * 128], bbp[:])

    def load_seg_dma(n, hg, eng=None):
        """Issue just the cast-DMA for one segment (SWDGE ring by default)."""
        c0 = hg * HG_CH
        seg = segpool.tile([128, HG_CH * WPAD], BF16, name="seg", tag="seg")
        segs[(n, hg)] = seg
        s3 = seg.rearrange("p (c w) -> p c w", c=HG_CH)
        return (eng or nc.gpsimd).dma_start(
            s3[:, :, 1:129],
            x_ap[n, c0:c0 + HG_CH, :, :].transpose([1, 0, 2]))

    def post_seg(n, hg):
        """Edge-fix + PE pooling (pass 1) for a loaded segment."""
        half = hg // 8
        s3 = segs[(n, hg)].rearrange("p (c w) -> p c w", c=HG_CH)
        nc.vector.tensor_copy(s3[:, :, 0:1], s3[:, :, 2:3])
        nc.vector.tensor_copy(s3[:, :, 129:130], s3[:, :, 127:128])
        if hg % 8 == 0:
            pq[(n, half, 0)] = [
                ppsum.tile([16, 512], F32, name="pq", tag="pp")
                for _ in P1Q]
        for i, q in enumerate(P1Q):
            nc.tensor.matmul(
                pq[(n, half, 0)][i][:],
                lhsT=selb_sb[:, hg * 16:(hg + 1) * 16],
                rhs=s3[:, q * 4:(q + 1) * 4, 1:129],
                start=(hg % 8 == 0), stop=(hg % 8 == 7))
        if hg % 8 == 7:
            pool_pass_reduce(n, half, 0)
            pool_pass2(n, half)
            filt_half(n, half)

    def load_image(n):
        for hg in range(N_HG):
            load_seg_dma(n, hg)
            post_seg(n, hg)

    g_built = {n: set() for n in range(N_PER_CORE)}

    def g_build(n, g):
        # G_dx = f0*D_up + f1*I + f2*D_dn per (g, dx); reflect rows in D mats
        if g in g_built[n]:
            return
        g_built[n].add(g)
        for dx in range(3):
            blk = gt[n][:, (g * 3 + dx) * 128:(g * 3 + dx + 1) * 128]
            j0 = g * 9 + 0 * 3 + dx
            j1 = g * 9 + 1 * 3 + dx
            j2 = g * 9 + 2 * 3 + dx
            nc.vector.tensor_scalar(
                out=blk, in0=dmbf_sb[:, 0:128],
                scalar1=fbs[n][:, j0:j0 + 1], scalar2=None, op0=ALU.mult)
            nc.vector.scalar_tensor_tensor(
                out=blk, in0=dmbf_sb[:, 128:256],
                scalar=fbs[n][:, j1:j1 + 1], in1=blk,
                op0=ALU.mult, op1=ALU.add)
            nc.vector.scalar_tensor_tensor(
                out=blk, in0=dmbf_sb[:, 256:384],
                scalar=fbs[n][:, j2:j2 + 1], in1=blk,
                op0=ALU.mult, op1=ALU.add)

    def conv_image(n, hgs=None):
        for hg in (range(N_HG) if hgs is None else hgs):
            c0 = hg * HG_CH
            g = c0 // CG
            seg = segs.pop((n, hg))
            s3 = seg.rearrange("p (c w) -> p c w", c=HG_CH)
            outst = opool.tile([128, HG_CH * W], F32, name="outst")
            outst3 = outst.rearrange("p (c w) -> p c w", c=HG_CH)
            xs = {}
            for q in range(4):
                xs1 = xspool.tile([128, 4 * WPAD], BF16, name="xs1")
                xs13 = xs1.rearrange("p (c w) -> p c w", c=4)
                for cc in range(4):
                    c = c0 + q * 4 + cc
                    ci = q * 4 + cc
                    nc.vector.tensor_scalar(
                        out=xs13[:, cc, :], in0=s3[:, ci, :],
                        scalar1=float(s1s2[c]), scalar2=None, op0=ALU.mult)
                xs[q] = xs13
            g_build(n, g)
            for q in range(4):
                xs13 = xs[q]
                ps = mpsum.tile([128, 512], F32, name="ps", tag="ps")
                for dx in range(3):
                    nc.tensor.matmul(
                        ps[:],
                        lhsT=gt[n][:, (g * 3 + dx) * 128:(g * 3 + dx + 1) * 128],
                        rhs=xs13[:, :, dx:dx + 128],
                        start=(dx == 0), stop=False)
                nc.tensor.matmul(
                    ps[:], lhsT=idnb,
                    rhs=s3[:, q * 4:(q + 1) * 4, 1:129],
                    start=False, stop=True)
                ps3 = ps.rearrange("p (c w) -> p c w", c=4)
                for cc in range(4):
                    ci = q * 4 + cc
                    c = c0 + ci
                    if ci % 8 == 0:
                        nc.vector.tensor_scalar(
                            out=outst3[:, ci, :], in0=ps3[:, cc, :],
                            scalar1=float(s2v[c]),
                            scalar2=b_n[n][:, c:c + 1],
                            op0=ALU.mult, op1=ALU.add)
                    else:
                        nc.scalar.activation(
                            outst3[:, ci, :], ps3[:, cc, :], AF.Identity,
                            bias=b_n[n][:, c:c + 1],
                            scale=float(s2v[c]))
            nc.sync.dma_start(out_ap[n, c0:c0 + HG_CH, :, :].transpose([1, 0, 2]),
                              outst3[:, :, :])

    LAG = 4
    load_image(0)
    for hg in range(N_HG):
        load_seg_dma(1, hg)
    for hg in range(N_HG):
        conv_image(0, [hg])
        if hg >= LAG:
            post_seg(1, hg - LAG)
    for hg in range(N_HG - LAG, N_HG):
        post_seg(1, hg)
    conv_image(1)


def build_nc(s1s2, s2v):
    nc = bacc.Bacc("TRN2", target_bir_lowering=False, debug=False)
    x_h = nc.dram_tensor("x", [N_PER_CORE, C, H, W], F32, kind="ExternalInput")
    dmbf_h = nc.dram_tensor("dmats_bf", [128, 384], BF16, kind="ExternalInput")
    sbc_h = nc.dram_tensor("sbc", [128, 512], F32, kind="ExternalInput")
    mbrow_h = nc.dram_tensor("mbrow", [1, 256], F32, kind="ExternalInput")
    wtd_h = nc.dram_tensor("wtd", [128, 144], F32, kind="ExternalInput")
    bnsb_h = nc.dram_tensor("bnsb", [72, 2], F32, kind="ExternalInput")
    selb_h = nc.dram_tensor("sel_bf", [128, 256], BF16, kind="ExternalInput")
    pb_h = nc.dram_tensor("pbounce", [N_PER_CORE, 256], F32, kind="Internal")
    out_h = nc.dram_tensor("out", [N_PER_CORE, C, H, W], F32, kind="ExternalOutput")

    with tile.TileContext(nc) as tc:
        with ExitStack() as ctx:
            _build_kernel(ctx, tc, x_h.ap(), out_h.ap(), pb_h.ap(),
                          dmbf_h.ap(), sbc_h.ap(), mbrow_h.ap(), wtd_h.ap(),
                          bnsb_h.ap(), selb_h.ap(), s1s2, s2v)
    nc.compile()
    return nc


def kernel(x, conv_w, bn_gamma, bn_beta, bn_mean, bn_var, lamb_l, lamb_h,
           inside_all, _trace=False, _trace_kwargs=None):
    x = np.ascontiguousarray(x, dtype=np.float32)
    consts, (s1s2, s2v) = _host_consts(conv_w, bn_gamma, bn_beta, bn_mean,
                                       bn_var, lamb_l, lamb_h, inside_all)
    nc = build_nc(s1s2, s2v)
    in_maps = []
    for i in range(N_CORES):
        m = {"x": x[i * N_PER_CORE:(i + 1) * N_PER_CORE]}
        m.update(consts)
        in_maps.append(m)
    kw = {}
    if _trace:
        kw["trace"] = True
        if _trace_kwargs:
            kw.update(_trace_kwargs)
    res = run_bass_kernel_spmd(nc, in_maps, list(range(N_CORES)), **kw)
    out = np.concatenate([res.results[i]["out"] for i in range(N_CORES)], axis=0)
    if _trace:
        kernel.last_results = res
    return out
